# revision 12
# baseline (speedup 1.0000x reference)
"""BitLinear (absmean-ternary weight x int8-absmax activation) on 8 trn2 cores.

out[b,s,o] = sum_i x_q[b,s,i] * w_q[o,i]
  w_q = clip(round(w / (mean|w| + 1e-8)), -1, 1) * mean|w|
  x_q = clip(round(x / s_row), -127, 127) * s_row,  s_row = max(max|row|/127, 1e-8)

Strategy: 2D tensor-parallel grid R x C = 4 x 2 over (x rows, out
features). Core c -> (rid = c // 2, cid = c % 2): x rows [rid*2048 ...),
w rows [cid*2048 ...), plus a distinct 512-row scale slice of w (8 cores
x 512 = full w) so the global absmean AllReduce fires after only 8.4 MB
of w reads. Per-core HBM ~92 MB (vs 151 replicated-x), transposes ~34 MB
(vs 88): tensor-bound.

Numerics (validated vs reference at 3.2e-5): quantized operands are
small exact integers (x_int in [-127,127] exact in bf16, w_t in {-1,0,1}
exact in fp8e4), dot products < 2^24 accumulate exactly in f32 PSUM;
scales applied on eviction. RNE via the fp32 magic-number (1.5*2^23).
The matmul runs mixed: bf16 stationary (x^T tile) x fp8 moving (w^T),
letting the full transposed ternary weight stay resident in SBUF (64
KB/partition as two [128,32,1024] fp8 tiles).

Schedule (from trace analysis):
 - dummy AllReduce at t=0 absorbs the CC-stream init barrier.
 - scale chain (slice loads, |w| reduces, AllReduce, consts) runs at
   priority 0 so the sim scheduler doesn't slot x work ahead of it.
 - w pipeline in 32 half-tiles ([128,2048] f32 loads, 4-deep) to cut
   per-stage latency; also priority-boosted. First matmul needs only
   w row-blocks 0-3 (chunk 0).
 - x pipeline (scale-independent) streams from t=0 on the sync queue
   (loads + ALL XBAR transposes live there; one queue only).
 - m-tiles 0-3: chunk-outer matmuls, 512-wide moving, start as soon as
   chunk 0 is converted. m-tiles 4+: k-outer with two 1024-wide MMs per
   LDWEIGHTS (stationary shared) to amortize the weight-load bubble.
 - evicts (PSUM * s_tot[m]) alternate ACT/DVE at high priority so PSUM
   banks recycle promptly.
"""

from contextlib import ExitStack, contextmanager

import numpy as np

import concourse.mybir as mybir
import concourse.tile as tile
from concourse import bacc, bass_isa
from concourse.bass_utils import run_bass_kernel_spmd

F32 = mybir.dt.float32
BF16 = mybir.dt.bfloat16
FP8 = mybir.dt.float8e4

MAGIC = 12582912.0  # 1.5 * 2^23: fp32 RNE rounder for |v| < 2^22
N_CORES = 8
P = 128
IN_F = 4096                  # contraction dim (i)
K_TILES = IN_F // P          # 32
ROWS_F = 8192
OUT_F = 4096

R = 4                        # row groups
C = 2                        # out-feature groups
SH_C = OUT_F // C            # out features per core (2048)
WR_BLOCKS = SH_C // P        # w row-blocks per core (16)
WHALF = IN_F // 2            # w load half width (2048)
CHUNK = 512                  # block-0 o-chunk width
CHUNKS = SH_C // CHUNK       # 4
BIG = 1024                   # k-outer moving width
BIGS = SH_C // BIG           # 2
WS_TILES = 4                 # scale-slice tiles (512 rows)
BLOCK = 4                    # m-tiles in the chunk-outer prologue block
XQT_BUFS = 5
DUMMY_CC = True
HALF = IN_F // 2             # cols of x's -MAGIC pass done on DVE

_MEAN_C = float(np.float32(2.0**-24))                    # 1/(4096*4096)
_EPS = float(np.float32(1e-8))
_SW127_C = float(np.float32(np.float32(2.0**-24) * np.float32(1.0 / 127.0)))


@contextmanager
def _hi(tc):
    with tc.high_priority():
        yield


def _body(ctx, tc, x_ap, w_ap, ws_ap, o_ap, m_tiles):
    nc = tc.nc

    const = ctx.enter_context(tc.tile_pool(name="const", bufs=1))
    dramp = ctx.enter_context(tc.tile_pool(name="dram", bufs=1, space="DRAM"))
    xp = ctx.enter_context(tc.tile_pool(name="x", bufs=2))
    wlp = ctx.enter_context(tc.tile_pool(name="wl", bufs=4))
    xqp = ctx.enter_context(tc.tile_pool(name="xq", bufs=2))
    xqtp = ctx.enter_context(tc.tile_pool(name="xqt", bufs=XQT_BUFS))
    wqtp = ctx.enter_context(tc.tile_pool(name="wqt", bufs=2))
    psa = ctx.enter_context(tc.tile_pool(name="psa", bufs=2, space="PSUM"))
    psb = ctx.enter_context(tc.tile_pool(name="psb", bufs=6, space="PSUM"))
    outp = ctx.enter_context(tc.tile_pool(name="out", bufs=2))
    statp = ctx.enter_context(tc.tile_pool(name="stat", bufs=max(m_tiles, 1)))

    # ---------------- CC stream warmup ----------------
    cc_in = dramp.tile([1, 1], F32)
    cc_out = dramp.tile([1, 1], F32)
    if DUMMY_CC:
        with _hi(tc):
            zt = const.tile([1, 1], F32)
            nc.gpsimd.memset(zt[:], 0.0)
            cc_in0 = dramp.tile([1, 1], F32)
            cc_out0 = dramp.tile([1, 1], F32)
            nc.gpsimd.dma_start(cc_in0[:], zt[:])
            nc.gpsimd.collective_compute(
                "AllReduce", mybir.AluOpType.add,
                replica_groups=[list(range(N_CORES))],
                ins=[cc_in0[:].opt()], outs=[cc_out0[:].opt()],
            )

    # ---------------- x quantization (two pipelined stages) ----------------
    stageA = {}
    stageB = {}
    mrows = {}
    s_tots = {}

    def x_quant_a(mt):
        x = xp.tile([P, IN_F], F32, tag="x")
        nc.sync.dma_start(x[:], x_ap[mt * P:(mt + 1) * P, :])
        mrow = statp.tile([P, 1], F32, tag="mrow")
        nc.vector.tensor_reduce(mrow[:], x[:], axis=mybir.AxisListType.X,
                                op=mybir.AluOpType.max,
                                apply_absolute_value=True)
        mrows[mt] = mrow
        r127 = statp.tile([P, 1], F32, tag="r127")
        nc.vector.reciprocal(r127[:], mrow[:])
        nc.vector.tensor_scalar_mul(r127[:], r127[:], 127.0)
        nc.scalar.activation(x[:], x[:], mybir.ActivationFunctionType.Copy,
                             bias=MAGIC, scale=r127[:])
        stageA[mt] = x

    def x_quant_b(mt):
        x = stageA.pop(mt)
        xq = xqp.tile([P, IN_F], BF16, tag="xq")
        nc.vector.tensor_scalar_sub(xq[:, :HALF], x[:, :HALF], MAGIC)
        nc.scalar.activation(xq[:, HALF:], x[:, HALF:],
                             mybir.ActivationFunctionType.Copy, bias=-MAGIC)
        xqT = xqtp.tile([P, K_TILES, P], BF16, tag="xqT")
        nc.sync.dma_start_transpose(xqT[:], xq[:])
        stageB[mt] = xqT

    def s_tot_of(mt):
        mrow = mrows.pop(mt)
        with _hi(tc):
            s_tot = statp.tile([P, 1], F32, tag="stot")
            nc.vector.tensor_tensor(s_tot[:], mrow[:], sw127[:],
                                    op=mybir.AluOpType.mult)
        s_tots[mt] = s_tot

    # ---------------- weight scale phase (priority 0) ----------------
    with _hi(tc):
        partials = const.tile([P, 2 * WS_TILES], F32)
        for t in range(2 * WS_TILES):
            r, h = divmod(t, 2)
            wst = wlp.tile([P, WHALF], F32, tag="wl")
            nc.gpsimd.dma_start(wst[:],
                                ws_ap[r * P:(r + 1) * P,
                                      h * WHALF:(h + 1) * WHALF])
            nc.vector.tensor_reduce(partials[:, t:t + 1], wst[:],
                                    axis=mybir.AxisListType.X,
                                    op=mybir.AluOpType.add,
                                    apply_absolute_value=True)
        p1 = const.tile([P, 1], F32)
        nc.vector.tensor_reduce(p1[:], partials[:], axis=mybir.AxisListType.X,
                                op=mybir.AluOpType.add)
        pa = const.tile([P, 1], F32)
        nc.gpsimd.partition_all_reduce(pa[:], p1[:], channels=P,
                                       reduce_op=bass_isa.ReduceOp.add)
        nc.gpsimd.dma_start(cc_in[:], pa[:1, :1])
        nc.gpsimd.collective_compute(
            "AllReduce", mybir.AluOpType.add,
            replica_groups=[list(range(N_CORES))],
            ins=[cc_in[:].opt()], outs=[cc_out[:].opt()],
        )
        gs1 = const.tile([1, 1], F32)
        nc.gpsimd.dma_start(gs1[:], cc_out[:])
        gsum = const.tile([P, 1], F32)
        nc.gpsimd.partition_broadcast(gsum[:], gs1[:])

        scale_eps = const.tile([P, 1], F32)
        nc.vector.tensor_scalar(scale_eps[:], gsum[:], _MEAN_C, _EPS,
                                op0=mybir.AluOpType.mult,
                                op1=mybir.AluOpType.add)
        rec_w = const.tile([P, 1], F32)
        nc.vector.reciprocal(rec_w[:], scale_eps[:])
        sw127 = const.tile([P, 1], F32)
        nc.vector.tensor_scalar_mul(sw127[:], gsum[:], _SW127_C)

    # ---------------- weight quantize pipeline (32 half-tiles) -------------
    # wT[g] holds w_q^T for o-features [g*1024, (g+1)*1024): [128i, 32k, 1024o]
    wT = [const.tile([P, K_TILES, BIG], FP8, name=f"wT{g}")
          for g in range(BIGS)]

    def w_quant(t):
        # t = 2*r + h: row-block r (128 out-features), i-half h
        r, h = divmod(t, 2)
        with _hi(tc):
            wt = wlp.tile([P, WHALF], F32, tag="wl")
            nc.scalar.dma_start(wt[:],
                                w_ap[r * P:(r + 1) * P,
                                     h * WHALF:(h + 1) * WHALF])
            nc.scalar.activation(wt[:], wt[:],
                                 mybir.ActivationFunctionType.Copy,
                                 bias=MAGIC, scale=rec_w[:])
            wq = xqp.tile([P, WHALF], BF16, tag="wq", bufs=2)
            nc.vector.tensor_scalar_sub(wq[:], wt[:], MAGIC)
            wqT = wqtp.tile([P, K_TILES // 2, P], BF16, tag="wqT")
            nc.sync.dma_start_transpose(wqT[:], wq[:])
            g, j = divmod(r, WR_BLOCKS // BIGS)
            nc.gpsimd.tensor_scalar(
                wT[g][:, h * (K_TILES // 2):(h + 1) * (K_TILES // 2),
                      j * P:(j + 1) * P],
                wqT[:], 1.0, -1.0,
                op0=mybir.AluOpType.min, op1=mybir.AluOpType.max)

    # ------------- pre-main: x ahead of w transposes -----------------------
    a_next = 0
    b_next = 0
    while b_next < min(XQT_BUFS, m_tiles):
        while a_next < min(b_next + 2, m_tiles):
            x_quant_a(a_next)
            a_next += 1
        x_quant_b(b_next)
        b_next += 1

    for t in range(2 * WR_BLOCKS):
        w_quant(t)

    evict_flip = [True]

    def evict_store(mt, ps, col0, width):
        ot = outp.tile([P, width], F32, tag=f"o{width}",
                       bufs=2)
        with _hi(tc):
            if evict_flip[0]:
                nc.scalar.activation(ot[:], ps[:],
                                     mybir.ActivationFunctionType.Copy,
                                     scale=s_tots[mt][:])
            else:
                nc.vector.tensor_scalar_mul(ot[:], ps[:], s_tots[mt][:])
            evict_flip[0] = not evict_flip[0]
            nc.gpsimd.dma_start(o_ap[mt * P:(mt + 1) * P, col0:col0 + width],
                                ot[:])

    # ---------------- prologue block: chunk-outer, 512-wide ----------------
    mts0 = list(range(min(BLOCK, m_tiles)))
    xqTs = {mt: stageB.pop(mt) for mt in mts0}
    for mt in mts0:
        s_tot_of(mt)
    for ci in range(CHUNKS):
        g, half = divmod(ci, 2)
        for mt in mts0:
            ps = psa.tile([P, CHUNK], F32, tag="psa")
            for k in range(K_TILES):
                nc.tensor.matmul(ps[:], xqTs[mt][:, k, :],
                                 wT[g][:, k, half * CHUNK:(half + 1) * CHUNK],
                                 start=(k == 0), stop=(k == K_TILES - 1))
            evict_store(mt, ps, ci * CHUNK, CHUNK)

    # ---------------- main: k-outer, shared stationary (4 MMs per LDW) -----
    for mt in range(BLOCK, m_tiles):
        while b_next < min(m_tiles, mt + 3):
            while a_next < min(b_next + 2, m_tiles):
                x_quant_a(a_next)
                a_next += 1
            x_quant_b(b_next)
            b_next += 1
        xqT = stageB.pop(mt)
        s_tot_of(mt)
        pss = [psb.tile([P, CHUNK], F32, tag="psb", name=f"ps{ci}")
               for ci in range(CHUNKS)]
        for k in range(K_TILES):
            for ci in range(CHUNKS):
                g, half = divmod(ci, 2)
                nc.tensor.matmul(pss[ci][:], xqT[:, k, :],
                                 wT[g][:, k,
                                       half * CHUNK:(half + 1) * CHUNK],
                                 start=(k == 0), stop=(k == K_TILES - 1))
        for ci in range(CHUNKS):
            evict_store(mt, pss[ci], ci * CHUNK, CHUNK)


_NC_CACHE = {}


def build_nc(m_tiles_local):
    key = m_tiles_local
    if key in _NC_CACHE:
        return _NC_CACHE[key]
    nc = bacc.Bacc("TRN2", target_bir_lowering=False, debug=False,
                   num_devices=N_CORES)
    rows = m_tiles_local * P
    x_dram = nc.dram_tensor("x_in", [rows, IN_F], F32, kind="ExternalInput")
    w_dram = nc.dram_tensor("w_in", [SH_C, IN_F], F32, kind="ExternalInput")
    ws_dram = nc.dram_tensor("ws_in", [WS_TILES * P, IN_F], F32,
                             kind="ExternalInput")
    o_dram = nc.dram_tensor("out", [rows, SH_C], F32, kind="ExternalOutput")
    with tile.TileContext(nc) as tc, ExitStack() as ctx:
        _body(ctx, tc, x_dram.ap(), w_dram.ap(), ws_dram.ap(), o_dram.ap(),
              m_tiles_local)
    nc.compile()
    _NC_CACHE[key] = nc
    return nc


def run_sharded(x2d, weight, m_tiles, trace=False):
    """x2d: [m_tiles*128, 4096] f32, weight: [4096, 4096] f32."""
    assert m_tiles % R == 0, f"m_tiles {m_tiles} must divide by R={R}"
    mtl = m_tiles // R
    sh_r = mtl * P
    nc = build_nc(mtl)
    ws_rows = WS_TILES * P
    in_maps = []
    for core in range(N_CORES):
        rid, cid = divmod(core, C)
        in_maps.append({
            "x_in": x2d[rid * sh_r:(rid + 1) * sh_r],
            "w_in": weight[cid * SH_C:(cid + 1) * SH_C],
            "ws_in": weight[core * ws_rows:(core + 1) * ws_rows],
        })
    res = run_bass_kernel_spmd(nc, in_maps, core_ids=list(range(N_CORES)),
                               trace=trace)
    out = np.empty((m_tiles * P, OUT_F), dtype=np.float32)
    for core in range(N_CORES):
        rid, cid = divmod(core, C)
        out[rid * sh_r:(rid + 1) * sh_r,
            cid * SH_C:(cid + 1) * SH_C] = res.results[core]["out"]
    return out, res


def kernel(x, weight):
    b, s, f = x.shape
    x2d = np.ascontiguousarray(x.reshape(b * s, f)).astype(np.float32,
                                                           copy=False)
    w = np.ascontiguousarray(weight).astype(np.float32, copy=False)
    out, _ = run_sharded(x2d, w, (b * s) // P)
    return out.reshape(b, s, OUT_F).astype(np.float32, copy=False)
